# revision 10
# baseline (speedup 1.0000x reference)
"""Trainium2 Bass kernel for CausalGNNLayer (per-node-type Linear, MoE-style routing).

Semantics (matching the reference):
    out[n, :] = x[n, :] @ W[node_types[n]].T + b[node_types[n]]
edge_index is unused by the op.

Strategy:
- Host-side routing-aware sharding: stable-sort nodes by type, split each
  type's node list into two halves -> 8 groups (4 types x 2 cores).
- bf16 operands with fp32 PSUM accumulation; bf16 output storage.  This
  halves both HBM directions vs fp32 and keeps rel-err ~3e-3.
- Weight-stationary matmul schedule (out.T layout): psum[o_blk, nodes] +=
  w[k, o_blk].T @ xT[k, nodes].  The stationary operand (a 128x128 tile of
  W.T) is reused across 4 consecutive 512-node chunks, so LDWEIGHTS is
  amortized 4x and hides under the moving stream; matmuls stay back-to-back
  at the tensor engine's streaming rate.
- PSUM mega-tiles [128, 4*512] (4 banks) double-buffered = all 8 banks.
- Drain (bias add + fp32->bf16 downcast) alternates between the Vector and
  the otherwise-idle Scalar engine; in out.T layout the bias is a
  per-partition scalar, which both engines support natively.
- All DMA descriptors are 4KB contiguous runs per partition.
- Host scatters the 8 bf16 output shards back into the full [N, 512] fp32
  output.
"""

import numpy as np
import ml_dtypes
from contextlib import ExitStack

import concourse.bass as bass
import concourse.mybir as mybir
import concourse.tile as tile
from concourse.bass_utils import run_bass_kernel_spmd

N_CORES = 8
IN_CH = 512
OUT_CH = 512
NUM_TYPES = 4
P_BLK = 128          # partition count
KT = IN_CH // P_BLK  # 4 contraction tiles
CHUNK_N = 512        # nodes per chunk == psum bank capacity (fp32)
SGRP = 4             # chunks per stationary-reuse group (psum = SGRP banks)
XBUFS = 10           # x-chunk prefetch depth
PSBUFS = 2           # psum mega-tile ring (2 x 4 banks = all 8)
OBUFS = 2            # output staging depth (one whole group each)
WARMUP_LDW = 30      # dummy LDWEIGHTS to ramp the PE p-state during DMA wait

# Set by test harness to capture HW profile; kernel works without it.
TRACE = False
LAST_RESULTS = None

_compile_cache: dict = {}

_legal_nop_counter = [0]


def _legalize_waits(nc: bass.Bass) -> None:
    """This walrus codegen only encodes ONE sync wait per engine instruction.
    Tile's scheduler attaches several.  Split: hoist all-but-one wait of any
    multi-wait instruction into preceding same-engine NoOps (one wait each) —
    semantically identical (the engine stalls on each wait in program order)."""
    for fn in nc.m.functions:
        for blk in fn.blocks:
            insts = blk.instructions
            out = []
            changed = False
            for inst in insts:
                si = inst.sync_info
                waits = list(si.on_wait) if si is not None and si.on_wait else []
                if len(waits) > 1:
                    changed = True
                    for w in waits[:-1]:
                        _legal_nop_counter[0] += 1
                        nop = mybir.InstNoOp(
                            name=f"waitsplit-{_legal_nop_counter[0]}",
                            ins=[],
                            outs=[],
                            engine=inst.engine,
                        )
                        nop.sync_info = mybir.SyncInfo(on_wait=[w], on_update=[])
                        out.append(nop)
                    inst.sync_info = mybir.SyncInfo(
                        on_wait=[waits[-1]], on_update=list(si.on_update or [])
                    )
                out.append(inst)
            if changed:
                blk.instructions = out


def _build_bass(P: int) -> bass.Bass:
    """One-core program: outT[512, P] = w.T @ xT (+bias), weight-stationary."""
    nc = bass.Bass("TRN2")
    f32 = mybir.dt.float32
    bf16 = mybir.dt.bfloat16

    nchunks = P // CHUNK_N
    OBLKS = OUT_CH // P_BLK  # 4

    xT = nc.dram_tensor("xT", [nchunks, P_BLK, KT, CHUNK_N], bf16, kind="ExternalInput")
    w = nc.dram_tensor("w", [IN_CH, OUT_CH], bf16, kind="ExternalInput")
    # bias2[p, oblk] = b[oblk*128 + p]
    bias2 = nc.dram_tensor("bias2", [P_BLK, OBLKS], f32, kind="ExternalInput")
    # outT[oblk, p, n] = out[n, oblk*128+p]; per-partition runs are contiguous
    # along nodes so group writes are 4KB descriptors.
    out = nc.dram_tensor(
        "out", [OBLKS, P_BLK, nchunks * CHUNK_N], bf16, kind="ExternalOutput"
    )

    w_v = w.ap().rearrange("(k p) o -> p k o", p=P_BLK)

    # stationary-reuse groups; graded sizes at the start so the x-DMA stream
    # (1.4us/chunk) keeps ahead of each group's front-loaded chunk needs
    # during the startup transient, and a size-1 group last for a short tail.
    sizes = [1, 2, 2, 3, 3]
    rem = nchunks - sum(sizes) - 1
    while rem > 0:
        s = min(SGRP, rem)
        sizes.append(s)
        rem -= s
    sizes.append(1)
    assert sum(sizes) == nchunks
    groups = []
    c0 = 0
    for s in sizes:
        groups.append((c0, s))
        c0 += s

    with ExitStack() as ctx:
        tc = ctx.enter_context(tile.TileContext(nc))
        wp = ctx.enter_context(tc.tile_pool(name="w", bufs=1))
        warmp = ctx.enter_context(tc.tile_pool(name="warm", bufs=1))
        bp = ctx.enter_context(tc.tile_pool(name="b", bufs=1))
        xp = ctx.enter_context(tc.tile_pool(name="x", bufs=XBUFS))
        pp = ctx.enter_context(tc.tile_pool(name="ps", bufs=PSBUFS, space="PSUM"))
        op = ctx.enter_context(tc.tile_pool(name="o", bufs=OBUFS))

        # PE p-state warmup: the tensor engine ramps 0.65->1.2->2.4 GHz with
        # ~3us of continuous busy; dummy weight loads keep it busy while the
        # first data DMAs land so real matmuls start at full clock.  A dummy
        # scalar-engine op forces its one-time ACT_TABLE_LOAD (~1.3us) to
        # happen here instead of inside the first drain.
        warm_sb = warmp.tile([P_BLK, P_BLK], bf16)
        nc.gpsimd.memset(warm_sb[:], 0)
        nc.scalar.add(warm_sb[0:1, 0:1], warm_sb[0:1, 0:1], 0.0)
        for _ in range(WARMUP_LDW):
            nc.tensor.ldweights(warm_sb[:])

        x_tiles: dict[int, object] = {}

        def fetch_chunk(c: int):
            if c not in x_tiles:
                t = xp.tile([P_BLK, KT, CHUNK_N], bf16, tag="x")
                nc.sync.dma_start(t[:], xT.ap()[c])
                x_tiles[c] = t

        w_sb = wp.tile([P_BLK, KT, OUT_CH], bf16)
        # issue order: chunk 0 (the big first-matmul dependency), then w, so
        # compute starts as early as possible.
        fetch_chunk(0)
        for k in range(KT):
            nc.sync.dma_start(w_sb[:, k, :], w_v[:, k, :])
        b_sb = bp.tile([P_BLK, OBLKS], f32)
        nc.sync.dma_start(b_sb[:], bias2.ap())

        drain_flip = [0]
        for gi, (c0, sz) in enumerate(groups):
            for ci in range(sz):
                fetch_chunk(c0 + ci)
            # prefetch the next group's chunks right away
            if gi + 1 < len(groups):
                nc0, nsz = groups[gi + 1]
                for ci in range(nsz):
                    fetch_chunk(nc0 + ci)
            o_sb = op.tile([P_BLK, OBLKS, sz * CHUNK_N], bf16, tag="o")
            for oblk in range(OBLKS):
                ps = pp.tile([P_BLK, sz * CHUNK_N], f32, tag="ps")
                for k in range(KT):
                    lhsT = w_sb[:, k, oblk * P_BLK : (oblk + 1) * P_BLK]
                    for ci in range(sz):
                        nc.tensor.matmul(
                            ps[:, ci * CHUNK_N : (ci + 1) * CHUNK_N],
                            lhsT=lhsT,
                            rhs=x_tiles[c0 + ci][:, k, :],
                            start=(k == 0),
                            stop=(k == KT - 1),
                        )
                bias_ap = b_sb[:, oblk : oblk + 1]
                if drain_flip[0] % 2 == 0:
                    nc.vector.tensor_scalar_add(o_sb[:, oblk, :], ps[:], bias_ap)
                else:
                    nc.scalar.add(o_sb[:, oblk, :], ps[:], bias_ap)
                drain_flip[0] += 1
            # one coalesced out-DMA per group (single trigger, 4KB runs)
            dst = out.ap()[:, :, c0 * CHUNK_N : (c0 + sz) * CHUNK_N].rearrange(
                "ob p n -> p ob n"
            )
            nc.sync.dma_start(dst, o_sb[:])
    _legalize_waits(nc)
    return nc


def _get_compiled(P: int) -> bass.Bass:
    if P not in _compile_cache:
        _compile_cache[P] = _build_bass(P)
    return _compile_cache[P]


def kernel(x, edge_index, node_types, W, b):
    global LAST_RESULTS
    x = np.asarray(x, dtype=np.float32)
    nt = np.asarray(node_types).astype(np.int64)
    W = np.asarray(W, dtype=np.float32)
    b = np.asarray(b, dtype=np.float32)
    N = x.shape[0]

    # Route nodes: stable sort by type, split each type across 2 cores.
    order = np.argsort(nt, kind="stable")
    counts = np.bincount(nt, minlength=NUM_TYPES)
    groups = []
    start = 0
    for t in range(NUM_TYPES):
        c = int(counts[t])
        idx = order[start : start + c]
        start += c
        h = (c + 1) // 2
        groups.append(idx[:h])
        groups.append(idx[h:])

    P = max(1, max(len(g) for g in groups))
    P = ((P + CHUNK_N - 1) // CHUNK_N) * CHUNK_N
    nchunks = P // CHUNK_N

    nc = _get_compiled(P)

    in_maps = []
    for gi, g in enumerate(groups):
        t = gi // 2
        xs = np.zeros((P, IN_CH), np.float32)
        if len(g):
            xs[: len(g)] = x[g]
        # [P, 512] -> [nchunks, 128, KT, CHUNK_N] with partition contiguous
        xt = np.ascontiguousarray(
            xs.T.reshape(KT, P_BLK, nchunks, CHUNK_N).transpose(2, 1, 0, 3)
        ).astype(ml_dtypes.bfloat16)
        in_maps.append(
            {
                "xT": xt,
                "w": np.ascontiguousarray(W[t].T).astype(ml_dtypes.bfloat16),
                "bias2": np.ascontiguousarray(b[t].reshape(4, P_BLK).T.astype(np.float32)),
            }
        )

    res = run_bass_kernel_spmd(nc, in_maps, list(range(N_CORES)), trace=TRACE)
    LAST_RESULTS = res

    out = np.empty((N, OUT_CH), np.float32)
    for gi, g in enumerate(groups):
        if len(g):
            # outT [4, 128, P] -> [P, 512] node-major
            o = res.results[gi]["out"].reshape(OUT_CH, P).T.astype(np.float32)
            out[g] = o[: len(g)]
    return out


# revision 14
# speedup vs baseline: 1.2123x; 1.2123x over previous
"""Trainium2 Bass kernel for CausalGNNLayer (per-node-type Linear, MoE-style routing).

Semantics (matching the reference):
    out[n, :] = x[n, :] @ W[node_types[n]].T + b[node_types[n]]
edge_index is unused by the op.

Strategy:
- Host-side routing-aware sharding: stable-sort nodes by type, split each
  type's node list into two halves -> 8 groups (4 types x 2 cores).
- bf16 operands with fp32 PSUM accumulation; bf16 output storage.  This
  halves both HBM directions vs fp32 and keeps rel-err ~3e-3.
- Weight-stationary matmul schedule (out.T layout): psum[o_blk, nodes] +=
  w[k, o_blk].T @ xT[k, nodes].  The stationary operand (a 128x128 tile of
  W.T) is reused across 4 consecutive 512-node chunks, so LDWEIGHTS is
  amortized 4x and hides under the moving stream; matmuls stay back-to-back
  at the tensor engine's streaming rate.
- PSUM mega-tiles [128, 4*512] (4 banks) double-buffered = all 8 banks.
- Drain (bias add + fp32->bf16 downcast) alternates between the Vector and
  the otherwise-idle Scalar engine; in out.T layout the bias is a
  per-partition scalar, which both engines support natively.
- All DMA descriptors are 4KB contiguous runs per partition.
- Host scatters the 8 bf16 output shards back into the full [N, 512] fp32
  output.
"""

import numpy as np
import ml_dtypes
from contextlib import ExitStack

import concourse.bass as bass
import concourse.mybir as mybir
import concourse.tile as tile
from concourse.bass_utils import run_bass_kernel_spmd

N_CORES = 8
IN_CH = 512
OUT_CH = 512
NUM_TYPES = 4
P_BLK = 128          # partition count
KT = IN_CH // P_BLK  # 4 contraction tiles
CHUNK_N = 512        # nodes per chunk == psum bank capacity (fp32)
SGRP = 4             # chunks per stationary-reuse group (psum = SGRP banks)
XBUFS = 10           # x-chunk prefetch depth
PSBUFS = 2           # psum mega-tile ring (2 x 4 banks = all 8)
OBUFS = 4            # output staging depth
WARMUP_LDW = 30      # dummy LDWEIGHTS to ramp the PE p-state during DMA wait

# Set by test harness to capture HW profile; kernel works without it.
TRACE = False
LAST_RESULTS = None

_compile_cache: dict = {}

_legal_nop_counter = [0]


def _legalize_waits(nc: bass.Bass) -> None:
    """This walrus codegen only encodes ONE sync wait per engine instruction.
    Tile's scheduler attaches several.  Split: hoist all-but-one wait of any
    multi-wait instruction into preceding same-engine NoOps (one wait each) —
    semantically identical (the engine stalls on each wait in program order)."""
    for fn in nc.m.functions:
        for blk in fn.blocks:
            insts = blk.instructions
            out = []
            changed = False
            for inst in insts:
                si = inst.sync_info
                waits = list(si.on_wait) if si is not None and si.on_wait else []
                if len(waits) > 1:
                    changed = True
                    for w in waits[:-1]:
                        _legal_nop_counter[0] += 1
                        nop = mybir.InstNoOp(
                            name=f"waitsplit-{_legal_nop_counter[0]}",
                            ins=[],
                            outs=[],
                            engine=inst.engine,
                        )
                        nop.sync_info = mybir.SyncInfo(on_wait=[w], on_update=[])
                        out.append(nop)
                    inst.sync_info = mybir.SyncInfo(
                        on_wait=[waits[-1]], on_update=list(si.on_update or [])
                    )
                out.append(inst)
            if changed:
                blk.instructions = out


def _build_bass(P: int) -> bass.Bass:
    """One-core program: outT[512, P] = w.T @ xT (+bias), weight-stationary."""
    nc = bass.Bass("TRN2")
    f32 = mybir.dt.float32
    bf16 = mybir.dt.bfloat16

    nchunks = P // CHUNK_N
    OBLKS = OUT_CH // P_BLK  # 4

    xT = nc.dram_tensor("xT", [nchunks, P_BLK, KT, CHUNK_N], bf16, kind="ExternalInput")
    w = nc.dram_tensor("w", [IN_CH, OUT_CH], bf16, kind="ExternalInput")
    # bias2[p, oblk] = b[oblk*128 + p]
    bias2 = nc.dram_tensor("bias2", [P_BLK, OBLKS], f32, kind="ExternalInput")
    # outT[oblk, p, n] = out[n, oblk*128+p]; per-partition runs are contiguous
    # along nodes so group writes are 4KB descriptors.
    out = nc.dram_tensor(
        "out", [OBLKS, P_BLK, nchunks * CHUNK_N], bf16, kind="ExternalOutput"
    )

    w_v = w.ap().rearrange("(k p) o -> p k o", p=P_BLK)

    # stationary-reuse groups; graded sizes at the start so the x-DMA stream
    # (1.4us/chunk) keeps ahead of each group's front-loaded chunk needs
    # during the startup transient, and a size-1 group last for a short tail.
    sizes = [1, 2, 2, 3, 3]
    rem = nchunks - sum(sizes) - 1
    while rem > 0:
        s = min(SGRP, rem)
        sizes.append(s)
        rem -= s
    sizes.append(1)
    assert sum(sizes) == nchunks
    groups = []
    c0 = 0
    for s in sizes:
        groups.append((c0, s))
        c0 += s

    with ExitStack() as ctx:
        tc = ctx.enter_context(tile.TileContext(nc))
        wp = ctx.enter_context(tc.tile_pool(name="w", bufs=1))
        warmp = ctx.enter_context(tc.tile_pool(name="warm", bufs=1))
        bp = ctx.enter_context(tc.tile_pool(name="b", bufs=1))
        xp = ctx.enter_context(tc.tile_pool(name="x", bufs=XBUFS))
        pp = ctx.enter_context(tc.tile_pool(name="ps", bufs=PSBUFS, space="PSUM"))
        op = ctx.enter_context(tc.tile_pool(name="o", bufs=OBUFS))

        # PE p-state warmup: the tensor engine ramps 0.65->1.2->2.4 GHz with
        # ~3us of continuous busy; dummy weight loads keep it busy while the
        # first data DMAs land so real matmuls start at full clock.  A dummy
        # scalar-engine op forces its one-time ACT_TABLE_LOAD (~1.3us) to
        # happen here instead of inside the first drain.
        warm_sb = warmp.tile([P_BLK, P_BLK + 1], bf16)
        nc.gpsimd.memset(warm_sb[:], 0)
        for _ in range(WARMUP_LDW):
            nc.tensor.ldweights(warm_sb[:, 0:P_BLK])
        # ACT-table preload (~1.3us one-time) so the first real drain on the
        # scalar engine isn't delayed; issued after the LDWEIGHTS warmup so it
        # cannot serialize ahead of it.
        nc.scalar.add(warm_sb[0:1, P_BLK : P_BLK + 1], warm_sb[0:1, P_BLK : P_BLK + 1], 0.0)

        x_tiles: dict[int, object] = {}

        def fetch_chunk(c: int):
            if c not in x_tiles:
                t = xp.tile([P_BLK, KT, CHUNK_N], bf16, tag="x")
                nc.sync.dma_start(t[:], xT.ap()[c])
                x_tiles[c] = t

        w_sb = wp.tile([P_BLK, KT, OUT_CH], bf16)
        # issue order: chunk 0 (the big first-matmul dependency), then w, so
        # compute starts as early as possible.
        fetch_chunk(0)
        for k in range(KT):
            nc.sync.dma_start(w_sb[:, k, :], w_v[:, k, :])
        b_sb = bp.tile([P_BLK, OBLKS], f32)
        nc.sync.dma_start(b_sb[:], bias2.ap())

        drain_flip = [0]
        for gi, (c0, sz) in enumerate(groups):
            for ci in range(sz):
                fetch_chunk(c0 + ci)
            # prefetch the next group's chunks right away
            if gi + 1 < len(groups):
                nc0, nsz = groups[gi + 1]
                for ci in range(nsz):
                    fetch_chunk(nc0 + ci)
            for oblk in range(OBLKS):
                ps = pp.tile([P_BLK, sz * CHUNK_N], f32, tag="ps")
                for k in range(KT):
                    lhsT = w_sb[:, k, oblk * P_BLK : (oblk + 1) * P_BLK]
                    for ci in range(sz):
                        nc.tensor.matmul(
                            ps[:, ci * CHUNK_N : (ci + 1) * CHUNK_N],
                            lhsT=lhsT,
                            rhs=x_tiles[c0 + ci][:, k, :],
                            start=(k == 0),
                            stop=(k == KT - 1),
                        )
                o_sb = op.tile([P_BLK, sz * CHUNK_N], bf16, tag="o")
                bias_ap = b_sb[:, oblk : oblk + 1]
                if drain_flip[0] % 2 == 0:
                    nc.vector.tensor_scalar_add(o_sb[:], ps[:], bias_ap)
                else:
                    nc.scalar.add(o_sb[:], ps[:], bias_ap)
                drain_flip[0] += 1
                nc.sync.dma_start(
                    out.ap()[oblk, :, c0 * CHUNK_N : (c0 + sz) * CHUNK_N], o_sb[:]
                )
    _legalize_waits(nc)
    return nc


def _get_compiled(P: int) -> bass.Bass:
    if P not in _compile_cache:
        _compile_cache[P] = _build_bass(P)
    return _compile_cache[P]


def kernel(x, edge_index, node_types, W, b):
    global LAST_RESULTS
    x = np.asarray(x, dtype=np.float32)
    nt = np.asarray(node_types).astype(np.int64)
    W = np.asarray(W, dtype=np.float32)
    b = np.asarray(b, dtype=np.float32)
    N = x.shape[0]

    # Route nodes: stable sort by type, split each type across 2 cores.
    order = np.argsort(nt, kind="stable")
    counts = np.bincount(nt, minlength=NUM_TYPES)
    groups = []
    start = 0
    for t in range(NUM_TYPES):
        c = int(counts[t])
        idx = order[start : start + c]
        start += c
        h = (c + 1) // 2
        groups.append(idx[:h])
        groups.append(idx[h:])

    P = max(1, max(len(g) for g in groups))
    P = ((P + CHUNK_N - 1) // CHUNK_N) * CHUNK_N
    nchunks = P // CHUNK_N

    nc = _get_compiled(P)

    in_maps = []
    for gi, g in enumerate(groups):
        t = gi // 2
        xs = np.zeros((P, IN_CH), np.float32)
        if len(g):
            xs[: len(g)] = x[g]
        # [P, 512] -> [nchunks, 128, KT, CHUNK_N] with partition contiguous
        xt = np.ascontiguousarray(
            xs.T.reshape(KT, P_BLK, nchunks, CHUNK_N).transpose(2, 1, 0, 3)
        ).astype(ml_dtypes.bfloat16)
        in_maps.append(
            {
                "xT": xt,
                "w": np.ascontiguousarray(W[t].T).astype(ml_dtypes.bfloat16),
                "bias2": np.ascontiguousarray(b[t].reshape(4, P_BLK).T.astype(np.float32)),
            }
        )

    res = run_bass_kernel_spmd(nc, in_maps, list(range(N_CORES)), trace=TRACE)
    LAST_RESULTS = res

    out = np.empty((N, OUT_CH), np.float32)
    for gi, g in enumerate(groups):
        if len(g):
            # outT [4, 128, P] -> [P, 512] node-major
            o = res.results[gi]["out"].reshape(OUT_CH, P).T.astype(np.float32)
            out[g] = o[: len(g)]
    return out
